# revision 11
# baseline (speedup 1.0000x reference)
"""BPR embedding-lookup kernel for 8 TRN2 NeuronCores.

Math (per batch element b):
    out[b] = dot(user_emb[users[b]], item_emb[items[b]])
           + sum_u social_weight[users[b], u] * dot(item_emb[items[b]], user_emb[u])

Sharding: sort batch by user index, split into 8 contiguous chunks of 512.
Core m receives the contiguous social_weight row range covering its chunk's
users (~1/8 of the table), so every row gather stays local. Output is
inverse-permuted on the host.

Per-core device pipeline (bt = 4 batch tiles of 128):
  - indirect-DMA gather bu, bi rows and the [128, 10000] social_weight rows
  - scores = bi @ user_emb.T via PE matmul (biT stationary, user_embT moving)
  - prod = sw_chunk * scores on DVE; ACT Copy+accum_out row-sums each chunk;
    final small DVE reduce folds pos + 20 chunk sums into the output.
"""

import sys

if "/opt/trn_rl_repo" not in sys.path:
    sys.path.insert(0, "/opt/trn_rl_repo")

import numpy as np

NUM_USERS = 10000
NUM_ITEMS = 100000
D = 64
B = 4096
NCORES = 8
BL = B // NCORES          # 512 batch elements per core
NBT = BL // 128           # 4 batch tiles of 128
UCHUNK = 500              # psum-bank-sized score chunk (f32 <= 512)
NUC = NUM_USERS // UCHUNK # 20

_PROGRAM_CACHE = {}
LAST_RESULTS = None


def _build_program(s_pad: int):
    from concourse import bacc, bass, mybir, tile

    f32 = mybir.dt.float32
    i32 = mybir.dt.int32
    mult = mybir.AluOpType.mult
    add = mybir.AluOpType.add
    Copy = mybir.ActivationFunctionType.Copy
    AxisX = mybir.AxisListType.X

    nc = bacc.Bacc("TRN2", target_bir_lowering=False, debug=False, num_devices=NCORES)
    uembT_d = nc.declare_dram_parameter("uembT", [D, NUM_USERS], f32, isOutput=False)
    uemb_d = nc.declare_dram_parameter("uemb", [NUM_USERS, D], f32, isOutput=False)
    iemb_d = nc.declare_dram_parameter("iemb", [NUM_ITEMS, D], f32, isOutput=False)
    sw_d = nc.declare_dram_parameter("sw", [s_pad, NUM_USERS], f32, isOutput=False)
    uidx_d = nc.declare_dram_parameter("uidx", [128, NBT], i32, isOutput=False)
    ugidx_d = nc.declare_dram_parameter("ugidx", [128, NBT], i32, isOutput=False)
    iidx_d = nc.declare_dram_parameter("iidx", [128, NBT], i32, isOutput=False)
    out_d = nc.declare_dram_parameter("out", [128, NBT], f32, isOutput=True)

    with tile.TileContext(nc) as tc:
        with (
            tc.tile_pool(name="const", bufs=1) as constp,
            tc.tile_pool(name="sw", bufs=2) as swp,
            tc.tile_pool(name="small", bufs=3) as smallp,
            tc.tile_pool(name="accs", bufs=2) as accsp,
            tc.tile_pool(name="prod", bufs=3) as prodp,
            tc.tile_pool(name="junk", bufs=2) as junkp,
            tc.tile_pool(name="psum", bufs=4, space="PSUM") as psump,
        ):
            uembT_t = constp.tile([D, NUM_USERS], f32)
            for uc in range(NUC):
                sl = slice(uc * UCHUNK, (uc + 1) * UCHUNK)
                nc.sync.dma_start(out=uembT_t[:, sl], in_=uembT_d[:, sl])
            uidx_t = constp.tile([128, NBT], i32)
            nc.sync.dma_start(out=uidx_t[:], in_=uidx_d[:])
            ugidx_t = constp.tile([128, NBT], i32)
            nc.sync.dma_start(out=ugidx_t[:], in_=ugidx_d[:])
            iidx_t = constp.tile([128, NBT], i32)
            nc.sync.dma_start(out=iidx_t[:], in_=iidx_d[:])
            out_stage = constp.tile([128, NBT], f32)

            for bt in range(NBT):
                bu = smallp.tile([128, D], f32, tag="bu")
                nc.gpsimd.indirect_dma_start(
                    out=bu[:],
                    out_offset=None,
                    in_=uemb_d[:],
                    in_offset=bass.IndirectOffsetOnAxis(
                        ap=ugidx_t[:, bt : bt + 1], axis=0
                    ),
                )
                bi = smallp.tile([128, D], f32, tag="bi")
                nc.gpsimd.indirect_dma_start(
                    out=bi[:],
                    out_offset=None,
                    in_=iemb_d[:],
                    in_offset=bass.IndirectOffsetOnAxis(
                        ap=iidx_t[:, bt : bt + 1], axis=0
                    ),
                )
                swt = swp.tile([128, NUM_USERS], f32, tag="sw")
                nc.gpsimd.indirect_dma_start(
                    out=swt[:],
                    out_offset=None,
                    in_=sw_d[:],
                    in_offset=bass.IndirectOffsetOnAxis(
                        ap=uidx_t[:, bt : bt + 1], axis=0
                    ),
                )

                biT = smallp.tile([D, 128], f32, tag="biT")
                for j in range(D // 32):
                    for i in range(128 // 32):
                        nc.vector.transpose(
                            out=biT[32 * j : 32 * (j + 1), 32 * i : 32 * (i + 1)],
                            in_=bi[32 * i : 32 * (i + 1), 32 * j : 32 * (j + 1)],
                        )

                # acc_stage col 0 = pos, cols 1..NUC = social chunk sums
                acc_stage = accsp.tile([128, NUC + 1], f32, tag="accs")

                prod_pos = smallp.tile([128, D], f32, tag="prodpos")
                nc.vector.tensor_tensor(out=prod_pos[:], in0=bu[:], in1=bi[:], op=mult)
                junk_pos = smallp.tile([128, D], f32, tag="junkpos")
                nc.scalar.activation(
                    junk_pos[:], prod_pos[:], Copy, accum_out=acc_stage[:, 0:1]
                )

                for uc in range(NUC):
                    ps = psump.tile([128, UCHUNK], f32)
                    nc.tensor.matmul(
                        out=ps[:],
                        lhsT=biT[:],
                        rhs=uembT_t[:, uc * UCHUNK : (uc + 1) * UCHUNK],
                        start=True,
                        stop=True,
                    )
                    prod = prodp.tile([128, UCHUNK], f32, tag="prod")
                    nc.vector.tensor_tensor(
                        out=prod[:],
                        in0=swt[:, uc * UCHUNK : (uc + 1) * UCHUNK],
                        in1=ps[:],
                        op=mult,
                    )
                    junk = junkp.tile([128, UCHUNK], f32, tag="junk")
                    nc.scalar.activation(
                        junk[:], prod[:], Copy, accum_out=acc_stage[:, uc + 1 : uc + 2]
                    )

                nc.vector.tensor_reduce(
                    out=out_stage[:, bt : bt + 1],
                    in_=acc_stage[:],
                    axis=AxisX,
                    op=add,
                )

            nc.sync.dma_start(out=out_d[:], in_=out_stage[:])

    nc.finalize()
    return nc


def kernel(user_emb, item_emb, social_weight, users, items):
    global LAST_RESULTS
    import os

    from concourse.bass_utils import run_bass_kernel_spmd

    user_emb = np.ascontiguousarray(np.asarray(user_emb, dtype=np.float32))
    item_emb = np.ascontiguousarray(np.asarray(item_emb, dtype=np.float32))
    social_weight = np.ascontiguousarray(np.asarray(social_weight, dtype=np.float32))
    users = np.asarray(users).astype(np.int64)
    items = np.asarray(items).astype(np.int64)

    order = np.argsort(users, kind="stable")
    users_s = users[order]
    items_s = items[order]

    los, spans = [], []
    for m in range(NCORES):
        seg = users_s[m * BL : (m + 1) * BL]
        lo = int(seg[0])
        hi = int(seg[-1]) + 1
        los.append(lo)
        spans.append(hi - lo)
    s_pad = max(spans)

    if s_pad not in _PROGRAM_CACHE:
        _PROGRAM_CACHE[s_pad] = _build_program(s_pad)
    nc = _PROGRAM_CACHE[s_pad]

    uembT = np.ascontiguousarray(user_emb.T)
    in_maps = []
    for m in range(NCORES):
        seg_ug = users_s[m * BL : (m + 1) * BL].astype(np.int32)
        seg_u = (users_s[m * BL : (m + 1) * BL] - los[m]).astype(np.int32)
        seg_i = items_s[m * BL : (m + 1) * BL].astype(np.int32)
        sw_shard = np.empty((s_pad, NUM_USERS), np.float32)
        sw_shard[: spans[m]] = social_weight[los[m] : los[m] + spans[m]]
        in_maps.append(
            {
                "uembT": uembT,
                "uemb": user_emb,
                "iemb": item_emb,
                "sw": sw_shard,
                "uidx": np.ascontiguousarray(seg_u.reshape(NBT, 128).T),
                "ugidx": np.ascontiguousarray(seg_ug.reshape(NBT, 128).T),
                "iidx": np.ascontiguousarray(seg_i.reshape(NBT, 128).T),
            }
        )

    trace = bool(os.environ.get("CC_KERNEL_TRACE"))
    tmpdir = os.environ.get("CC_TRACE_DIR") or None
    res = run_bass_kernel_spmd(
        nc, in_maps, list(range(NCORES)), trace=trace, tmpdir=tmpdir
    )
    LAST_RESULTS = res

    out_sorted = np.empty(B, np.float32)
    for m in range(NCORES):
        o = np.asarray(res.results[m]["out"])  # [128, NBT]
        out_sorted[m * BL : (m + 1) * BL] = o.T.reshape(-1)

    final = np.empty(B, np.float32)
    final[order] = out_sorted
    return final


# revision 12
# speedup vs baseline: 1.9890x; 1.9890x over previous
"""BPR embedding-lookup kernel for 8 TRN2 NeuronCores.

Math (per batch element b):
    out[b] = dot(user_emb[users[b]], item_emb[items[b]])
           + sum_u social_weight[users[b], u] * dot(item_emb[items[b]], user_emb[u])

Reformulated per element as a single 64-length dot:
    out[b] = sum_d biT[d,b] * (V[d,b] + buT[d,b]),
    V[:,b] = user_emb.T @ social_weight[users[b], :]     (PE-accumulated)

Sharding: sort batch by user index, split into 8 contiguous chunks of 512.
Core m receives the contiguous social_weight row range covering its chunk's
users (~1/8 of the table) so row gathers stay local; output is
inverse-permuted on the host.

Per-core device pipeline:
  - social_weight rows arrive TRANSPOSED in SBUF via gpsimd dma_gather
    (transpose=True, bf16): partition = u%128, free = (u//128, b).
    This feeds the PE directly: 79 accumulating matmuls per 128-batch block
    with user_emb k-chunks stationary produce V^T [64, 512] in PSUM —
    no elementwise multiply or reduction pass over the [B,U] block at all.
  - bu rows arrive transposed the same way; bi rows (item ids exceed int16)
    use indirect DMA + DVE 32x32 block transposes.
  - tail: tmp = biT * (V^T + buT) on DVE, ones-matmul folds the d-reduction
    (pos + social together), ACT copies PSUM out.
"""

import sys

if "/opt/trn_rl_repo" not in sys.path:
    sys.path.insert(0, "/opt/trn_rl_repo")

import numpy as np

NUM_USERS = 10000
NUM_ITEMS = 100000
D = 64
B = 4096
NCORES = 8
BL = B // NCORES          # 512 batch elements per core
UK = 10112                # num_users padded to 79*128 (dma_gather needs 256B elems)
KC = UK // 128            # 79 contraction chunks
NG = 4                    # batch gather blocks per core
GB = BL // NG             # 128 indices per dma_gather (min granularity)

_PROGRAM_CACHE = {}
LAST_RESULTS = None


def _build_program(s_pad: int):
    import ml_dtypes  # noqa: F401

    from concourse import bacc, bass, mybir, tile

    f32 = mybir.dt.float32
    bf16 = mybir.dt.bfloat16
    i16 = mybir.dt.int16
    i32 = mybir.dt.int32
    mult = mybir.AluOpType.mult
    add = mybir.AluOpType.add

    nc = bacc.Bacc("TRN2", target_bir_lowering=False, debug=False, num_devices=NCORES)
    swp_d = nc.declare_dram_parameter("swp", [s_pad, UK], bf16, isOutput=False)
    uembk_d = nc.declare_dram_parameter("uembk", [UK, D], bf16, isOutput=False)
    uemb128_d = nc.declare_dram_parameter("uemb128", [NUM_USERS, 128], bf16, isOutput=False)
    iemb_d = nc.declare_dram_parameter("iemb", [NUM_ITEMS, D], f32, isOutput=False)
    swidx_d = nc.declare_dram_parameter("swidx", [128, BL // 16], i16, isOutput=False)
    ugidx16_d = nc.declare_dram_parameter("ugidx16", [128, BL // 16], i16, isOutput=False)
    iidx_d = nc.declare_dram_parameter("iidx", [128, NG], i32, isOutput=False)
    out_d = nc.declare_dram_parameter("out", [1, BL], f32, isOutput=True)

    with tile.TileContext(nc) as tc:
        with (
            tc.tile_pool(name="const", bufs=1) as constp,
            tc.tile_pool(name="swt", bufs=2) as swtp,
            tc.tile_pool(name="small", bufs=3) as smallp,
            tc.tile_pool(name="psum", bufs=1, space="PSUM") as psump,
            tc.tile_pool(name="psum2", bufs=1, space="PSUM") as psum2p,
        ):
            uembk_t = constp.tile([128, KC, D], bf16)
            nc.sync.dma_start(
                out=uembk_t[:], in_=uembk_d[:].rearrange("(c p) d -> p c d", p=128)
            )
            swidx_t = constp.tile([128, BL // 16], i16)
            nc.sync.dma_start(out=swidx_t[:], in_=swidx_d[:])
            ugidx_t = constp.tile([128, BL // 16], i16)
            nc.sync.dma_start(out=ugidx_t[:], in_=ugidx16_d[:])
            iidx_t = constp.tile([128, NG], i32)
            nc.sync.dma_start(out=iidx_t[:], in_=iidx_d[:])
            ones_t = constp.tile([D, 1], f32)
            nc.gpsimd.memset(ones_t[:], 1.0)

            # buT[d, b] = user_emb[users[b], d] (partitions 64.. zero-padded)
            buT = constp.tile([128, 1, BL], bf16)
            nc.gpsimd.dma_gather(
                out_ap=buT[:],
                in_ap=uemb128_d[:],
                idxs_ap=ugidx_t[:],
                num_idxs=BL,
                num_idxs_reg=BL,
                elem_size=128,
                transpose=True,
            )

            # biT[d, b] = item_emb[items[b], d] via indirect gather + DVE transpose
            biT = constp.tile([D, BL], f32)
            for g in range(NG):
                bi = smallp.tile([128, D], f32, tag="bi")
                nc.gpsimd.indirect_dma_start(
                    out=bi[:],
                    out_offset=None,
                    in_=iemb_d[:],
                    in_offset=bass.IndirectOffsetOnAxis(ap=iidx_t[:, g : g + 1], axis=0),
                )
                for j in range(D // 32):
                    for i in range(128 // 32):
                        nc.vector.transpose(
                            out=biT[32 * j : 32 * (j + 1), g * 128 + 32 * i : g * 128 + 32 * (i + 1)],
                            in_=bi[32 * i : 32 * (i + 1), 32 * j : 32 * (j + 1)],
                        )

            # V^T[d, b] accumulated over 79 k-chunks per 128-batch block
            vt_ps = psump.tile([D, BL], f32)
            for g in range(NG):
                swt = swtp.tile([128, KC, GB], bf16, tag="swt")
                nc.gpsimd.dma_gather(
                    out_ap=swt[:],
                    in_ap=swp_d[:],
                    idxs_ap=swidx_t[:, g * (GB // 16) : (g + 1) * (GB // 16)],
                    num_idxs=GB,
                    num_idxs_reg=GB,
                    elem_size=UK,
                    transpose=True,
                )
                for c in range(KC):
                    nc.tensor.matmul(
                        out=vt_ps[:, g * GB : (g + 1) * GB],
                        lhsT=uembk_t[:, c, :],
                        rhs=swt[:, c, :],
                        start=(c == 0),
                        stop=(c == KC - 1),
                    )

            tmp2 = constp.tile([D, BL], f32)
            nc.vector.tensor_tensor(out=tmp2[:], in0=vt_ps[:], in1=buT[:D, 0, :], op=add)
            tmp3 = constp.tile([D, BL], f32)
            nc.vector.tensor_tensor(out=tmp3[:], in0=tmp2[:], in1=biT[:], op=mult)
            res_ps = psum2p.tile([1, BL], f32)
            nc.tensor.matmul(
                out=res_ps[:], lhsT=ones_t[:], rhs=tmp3[:], start=True, stop=True
            )
            res_t = constp.tile([1, BL], f32)
            nc.scalar.copy(out=res_t[:], in_=res_ps[:])
            nc.sync.dma_start(out=out_d[:], in_=res_t[:])

    nc.finalize()
    return nc


def _wrap16(idx):
    """[BL] int -> [128, BL//16] int16: idx i at (i%16, i//16), replicated x8."""
    n = len(idx)
    blk = np.empty((16, n // 16), np.int16)
    blk[np.arange(n) % 16, np.arange(n) // 16] = idx.astype(np.int16)
    return np.ascontiguousarray(np.tile(blk, (8, 1)))


def kernel(user_emb, item_emb, social_weight, users, items):
    global LAST_RESULTS
    import os

    import ml_dtypes

    from concourse.bass_utils import run_bass_kernel_spmd

    bf = ml_dtypes.bfloat16
    user_emb = np.ascontiguousarray(np.asarray(user_emb, dtype=np.float32))
    item_emb = np.ascontiguousarray(np.asarray(item_emb, dtype=np.float32))
    social_weight = np.ascontiguousarray(np.asarray(social_weight, dtype=np.float32))
    users = np.asarray(users).astype(np.int64)
    items = np.asarray(items).astype(np.int64)

    order = np.argsort(users, kind="stable")
    users_s = users[order]
    items_s = items[order]

    los, spans = [], []
    for m in range(NCORES):
        seg = users_s[m * BL : (m + 1) * BL]
        lo = int(seg[0])
        hi = int(seg[-1]) + 1
        los.append(lo)
        spans.append(hi - lo)
    s_pad = max(spans)

    if s_pad not in _PROGRAM_CACHE:
        _PROGRAM_CACHE[s_pad] = _build_program(s_pad)
    nc = _PROGRAM_CACHE[s_pad]

    uembk = np.zeros((UK, D), bf)
    uembk[:NUM_USERS] = user_emb.astype(bf)
    uemb128 = np.zeros((NUM_USERS, 128), bf)
    uemb128[:, :D] = user_emb.astype(bf)

    in_maps = []
    for m in range(NCORES):
        seg_ug = users_s[m * BL : (m + 1) * BL]
        seg_u = (seg_ug - los[m]).astype(np.int64)
        seg_i = items_s[m * BL : (m + 1) * BL].astype(np.int32)
        swp = np.zeros((s_pad, UK), bf)
        swp[: spans[m], :NUM_USERS] = social_weight[los[m] : los[m] + spans[m]].astype(
            bf
        )
        in_maps.append(
            {
                "swp": swp,
                "uembk": uembk,
                "uemb128": uemb128,
                "iemb": item_emb,
                "swidx": _wrap16(seg_u),
                "ugidx16": _wrap16(seg_ug),
                "iidx": np.ascontiguousarray(seg_i.reshape(NG, 128).T),
            }
        )

    trace = bool(os.environ.get("CC_KERNEL_TRACE"))
    tmpdir = os.environ.get("CC_TRACE_DIR") or None
    res = run_bass_kernel_spmd(
        nc, in_maps, list(range(NCORES)), trace=trace, tmpdir=tmpdir
    )
    LAST_RESULTS = res

    out_sorted = np.empty(B, np.float32)
    for m in range(NCORES):
        out_sorted[m * BL : (m + 1) * BL] = np.asarray(res.results[m]["out"])[0]

    final = np.empty(B, np.float32)
    final[order] = out_sorted
    return final


# revision 16
# speedup vs baseline: 2.5137x; 1.2638x over previous
"""BPR embedding-lookup kernel for 8 TRN2 NeuronCores.

Math (per batch element b):
    out[b] = dot(user_emb[users[b]], item_emb[items[b]])
           + sum_u social_weight[users[b], u] * dot(item_emb[items[b]], user_emb[u])

Reformulated per element as a single 64-length dot:
    out[b] = sum_d biT[d,b] * (V[d,b] + buT[d,b]),
    V[:,b] = user_emb.T @ social_weight[users[b], :]     (PE-accumulated)

Sharding: sort batch by user index, split into 8 contiguous chunks of 512.
Core m receives the contiguous social_weight row range covering its chunk's
users (~1/8 of the table) so row gathers stay local; output is
inverse-permuted on the host.

Per-core device pipeline:
  - social_weight rows arrive TRANSPOSED in SBUF via gpsimd dma_gather
    (transpose=True, bf16): partition = u%128, free = (u//128, b).
    This feeds the PE directly: 79 accumulating matmuls per 128-batch block
    with user_emb k-chunks stationary produce V^T [64, 512] in PSUM —
    no elementwise multiply or reduction pass over the [B,U] block at all.
  - bu rows arrive transposed the same way; bi rows (item ids exceed int16)
    use indirect DMA + DVE 32x32 block transposes.
  - tail: tmp = biT * (V^T + buT) on DVE, ones-matmul folds the d-reduction
    (pos + social together), ACT copies PSUM out.
"""

import sys

if "/opt/trn_rl_repo" not in sys.path:
    sys.path.insert(0, "/opt/trn_rl_repo")

import numpy as np

NUM_USERS = 10000
NUM_ITEMS = 100000
D = 64
B = 4096
NCORES = 8
BL = B // NCORES          # 512 batch elements per core
UK = 10112                # num_users padded to 79*128 (dma_gather needs 256B elems)
KC = UK // 128            # 79 contraction chunks
NG = 4                    # batch gather blocks per core
GB = BL // NG             # 128 indices per dma_gather (min granularity)

_PROGRAM_CACHE = {}
LAST_RESULTS = None


def _build_program(s_pad: int):
    import ml_dtypes  # noqa: F401

    from concourse import bacc, bass, mybir, tile

    f32 = mybir.dt.float32
    bf16 = mybir.dt.bfloat16
    i16 = mybir.dt.int16
    i32 = mybir.dt.int32
    mult = mybir.AluOpType.mult
    add = mybir.AluOpType.add

    nc = bacc.Bacc(
        "TRN2",
        target_bir_lowering=False,
        debug=False,
        num_devices=NCORES,
        num_swdge_queues=4,
    )
    swp_d = nc.declare_dram_parameter("swp", [s_pad, UK], bf16, isOutput=False)
    # pre-arranged on host: uembk[p, c*D + d] = user_emb_padded[c*128 + p, d]
    uembk_d = nc.declare_dram_parameter("uembk", [128, KC * D], bf16, isOutput=False)
    uemb128_d = nc.declare_dram_parameter("uemb128", [NUM_USERS, 128], bf16, isOutput=False)
    iemb_d = nc.declare_dram_parameter("iemb", [NUM_ITEMS, D], f32, isOutput=False)
    swidx_d = nc.declare_dram_parameter("swidx", [128, BL // 16], i16, isOutput=False)
    ugidx16_d = nc.declare_dram_parameter("ugidx16", [128, BL // 16], i16, isOutput=False)
    iidx_d = nc.declare_dram_parameter("iidx", [128, NG], i32, isOutput=False)
    out_d = nc.declare_dram_parameter("out", [1, BL], f32, isOutput=True)

    with tile.TileContext(nc) as tc:
        with (
            tc.tile_pool(name="const", bufs=1) as constp,
            tc.tile_pool(name="swt", bufs=1) as swtp,
            tc.tile_pool(name="small", bufs=3) as smallp,
            tc.tile_pool(name="psum", bufs=1, space="PSUM") as psump,
            tc.tile_pool(name="psum2", bufs=1, space="PSUM") as psum2p,
        ):
            swidx_t = constp.tile([128, BL // 16], i16)
            nc.sync.dma_start(out=swidx_t[:], in_=swidx_d[:])
            ugidx_t = constp.tile([128, BL // 16], i16)
            nc.sync.dma_start(out=ugidx_t[:], in_=ugidx16_d[:])
            iidx_t = constp.tile([128, NG], i32)
            nc.sync.dma_start(out=iidx_t[:], in_=iidx_d[:])

            # social_weight transposed gathers first: one SWDGE queue each so
            # descriptor generation and drains overlap across blocks
            swts = []
            for g in range(NG):
                swt = swtp.tile([128, KC, GB], bf16, tag=f"swt{g}")
                nc.gpsimd.dma_gather(
                    out_ap=swt[:],
                    in_ap=swp_d[:],
                    idxs_ap=swidx_t[:, g * (GB // 16) : (g + 1) * (GB // 16)],
                    num_idxs=GB,
                    num_idxs_reg=GB,
                    elem_size=UK,
                    transpose=True,
                    queue_num=g % 4,
                )
                swts.append(swt)

            # buT[d, b] = user_emb[users[b], d] (partitions 64.. zero-padded)
            buT = constp.tile([128, 1, BL], bf16)
            nc.gpsimd.dma_gather(
                out_ap=buT[:],
                in_ap=uemb128_d[:],
                idxs_ap=ugidx_t[:],
                num_idxs=BL,
                num_idxs_reg=BL,
                elem_size=128,
                transpose=True,
            )

            uembk_t = constp.tile([128, KC, D], bf16)
            nc.sync.dma_start(
                out=uembk_t[:], in_=uembk_d[:].rearrange("p (c d) -> p c d", d=D)
            )
            ones_t = constp.tile([D, 1], f32)
            nc.vector.memset(ones_t[:], 1.0)

            # biT[d, b] = item_emb[items[b], d] via indirect gather + DVE transpose
            biT = constp.tile([D, BL], f32)
            for g in range(NG):
                bi = smallp.tile([128, D], f32, tag="bi")
                nc.gpsimd.indirect_dma_start(
                    out=bi[:],
                    out_offset=None,
                    in_=iemb_d[:],
                    in_offset=bass.IndirectOffsetOnAxis(ap=iidx_t[:, g : g + 1], axis=0),
                )
                for j in range(D // 32):
                    for i in range(128 // 32):
                        nc.vector.transpose(
                            out=biT[32 * j : 32 * (j + 1), g * 128 + 32 * i : g * 128 + 32 * (i + 1)],
                            in_=bi[32 * i : 32 * (i + 1), 32 * j : 32 * (j + 1)],
                        )

            # V^T[d, b] accumulated over 79 k-chunks per 128-batch block
            vt_ps = psump.tile([D, BL], f32)
            for g in range(NG):
                for c in range(KC):
                    nc.tensor.matmul(
                        out=vt_ps[:, g * GB : (g + 1) * GB],
                        lhsT=uembk_t[:, c, :],
                        rhs=swts[g][:, c, :],
                        start=(c == 0),
                        stop=(c == KC - 1),
                    )

            tmp2 = constp.tile([D, BL], f32)
            nc.vector.tensor_tensor(out=tmp2[:], in0=vt_ps[:], in1=buT[:D, 0, :], op=add)
            tmp3 = constp.tile([D, BL], f32)
            nc.vector.tensor_tensor(out=tmp3[:], in0=tmp2[:], in1=biT[:], op=mult)
            res_ps = psum2p.tile([1, BL], f32)
            nc.tensor.matmul(
                out=res_ps[:], lhsT=ones_t[:], rhs=tmp3[:], start=True, stop=True
            )
            res_t = constp.tile([1, BL], f32)
            nc.scalar.copy(out=res_t[:], in_=res_ps[:])
            nc.sync.dma_start(out=out_d[:], in_=res_t[:])

    nc.finalize()
    return nc


def _wrap16(idx):
    """[BL] int -> [128, BL//16] int16: idx i at (i%16, i//16), replicated x8."""
    n = len(idx)
    blk = np.empty((16, n // 16), np.int16)
    blk[np.arange(n) % 16, np.arange(n) // 16] = idx.astype(np.int16)
    return np.ascontiguousarray(np.tile(blk, (8, 1)))


def kernel(user_emb, item_emb, social_weight, users, items):
    global LAST_RESULTS
    import os

    import ml_dtypes

    from concourse.bass_utils import run_bass_kernel_spmd

    bf = ml_dtypes.bfloat16
    user_emb = np.ascontiguousarray(np.asarray(user_emb, dtype=np.float32))
    item_emb = np.ascontiguousarray(np.asarray(item_emb, dtype=np.float32))
    social_weight = np.ascontiguousarray(np.asarray(social_weight, dtype=np.float32))
    users = np.asarray(users).astype(np.int64)
    items = np.asarray(items).astype(np.int64)

    order = np.argsort(users, kind="stable")
    users_s = users[order]
    items_s = items[order]

    los, spans = [], []
    for m in range(NCORES):
        seg = users_s[m * BL : (m + 1) * BL]
        lo = int(seg[0])
        hi = int(seg[-1]) + 1
        los.append(lo)
        spans.append(hi - lo)
    s_pad = max(spans)

    if s_pad not in _PROGRAM_CACHE:
        _PROGRAM_CACHE[s_pad] = _build_program(s_pad)
    nc = _PROGRAM_CACHE[s_pad]

    uembk_pad = np.zeros((UK, D), bf)
    uembk_pad[:NUM_USERS] = user_emb.astype(bf)
    # [128, KC*D] with uembk[p, c*D+d] = uemb_pad[c*128+p, d]
    uembk = np.ascontiguousarray(
        uembk_pad.reshape(KC, 128, D).transpose(1, 0, 2).reshape(128, KC * D)
    )
    uemb128 = np.zeros((NUM_USERS, 128), bf)
    uemb128[:, :D] = user_emb.astype(bf)

    in_maps = []
    for m in range(NCORES):
        seg_ug = users_s[m * BL : (m + 1) * BL]
        seg_u = (seg_ug - los[m]).astype(np.int64)
        seg_i = items_s[m * BL : (m + 1) * BL].astype(np.int32)
        swp = np.zeros((s_pad, UK), bf)
        swp[: spans[m], :NUM_USERS] = social_weight[los[m] : los[m] + spans[m]].astype(
            bf
        )
        in_maps.append(
            {
                "swp": swp,
                "uembk": uembk,
                "uemb128": uemb128,
                "iemb": item_emb,
                "swidx": _wrap16(seg_u),
                "ugidx16": _wrap16(seg_ug),
                "iidx": np.ascontiguousarray(seg_i.reshape(NG, 128).T),
            }
        )

    trace = bool(os.environ.get("CC_KERNEL_TRACE"))
    tmpdir = os.environ.get("CC_TRACE_DIR") or None
    res = run_bass_kernel_spmd(
        nc, in_maps, list(range(NCORES)), trace=trace, tmpdir=tmpdir
    )
    LAST_RESULTS = res

    out_sorted = np.empty(B, np.float32)
    for m in range(NCORES):
        out_sorted[m * BL : (m + 1) * BL] = np.asarray(res.results[m]["out"])[0]

    final = np.empty(B, np.float32)
    final[order] = out_sorted
    return final
